# revision 39
# baseline (speedup 1.0000x reference)
"""CRF (linear-chain) loss kernel for Trainium2, 8-core data-parallel over batch.

Problem: emissions (512,1024,48) f32, tags (512,1024) i32, mask all-ones,
transitions (48,48), start/end (48,). Output: scalar mean loss.

Algorithm (per core, 64 batch rows):
  The log-partition (denominator) uses a *forward-backward split*: the
  forward recursion alpha runs from step 0 to the midpoint while the
  independent backward recursion gamma runs from step 1023 down to the
  midpoint; Z_b = sum_t alpha[t,b] * (W_b^T gamma)[t,b].  Both chains run
  in the *linear* domain, p <- exp(em) * (M^T p), with the transition
  matrices pre-scaled by exp(-MU) so per-step growth stays near 1; every
  R steps the per-column sums z are folded out (p *= 1/z, ln z recorded),
  applied DEFER steps late to stay off the critical path; all ln z are
  taken in one batched ACT Ln at the end.

  Layout: the F and B chains are STACKED ON PARTITIONS -- F tags on
  partitions 0-47, B tags on 64-111 (engine APs must start at 0/32/64/96;
  rows 48-63 are dead) -- with a block-diagonal 112x112 stationary
  [[Wf,0],[0,Wb]], so one PE matmul advances both chains.  The 64 batch
  columns are split into two groups of 32 whose dependency chains
  interleave on the engines, hiding the per-step PE->DVE->PE latency.

  The wall time is latency-bound: 511 sequential roundtrips of
  ~551ns each (matmul PSUM-write latency 173 + sem hops ~78 + DVE
  multiply busy 158 + DVE write-ack 125 + front-ends), so the design
  keeps the DVE stream free of everything except the two per-step chain
  multiplies:
    - chain state/stationaries are bf16 (PSUM accumulation stays fp32);
      MU is calibrated (4.9) so the linear-domain product drifts ~e^0
      per step and only 3 renorm events per chain are needed (R=128);
    - numerator emission-select runs on GPSIMD from a host-built one-hot
      tag mask (pure index data): prod = mask*em accumulated into wide
      fp32 rows, summed once at the end on ACT via accum_out;
    - renorm z-copies (PSUM->SBUF) and exp(em) slices run on ACT;
    - chunk-0's emissions DMA is issued first with a small head piece so
      the chain starts ~4.5us into the program; the parameter tables are
      packed into two staged tensors (one DMA each);
    - late-ready finalize ops carry an artificial dependency on the
      final chain state so the static scheduler cannot hoist them ahead
      of pending chain work in an in-order engine stream.
  The transition/start/end numerator contributions use host-side integer
  histograms of the tags (index statistics only) dotted with the
  parameter tables on device.
"""

import numpy as np

B, S, T = 512, 1024, 48
NCORES = 8
BL = B // NCORES          # 64 batch rows per core
NG = 2                    # batch groups (interleaved dependency chains)
GW = BL // NG             # 32 batch columns per group
OFF = 64                  # partition offset of the backward chain
P2 = OFF + T              # 112 partitions used; rows 48-63 are dead (zero)
MU = 4.9                  # per-step constant shift folded into the matrices
                          # (calibrated so per-step growth ~e^0; see sim_check)
R = 128                   # renormalize every R steps
DEFER = 4                 # apply the renorm scale this many steps late
CHUNK = 64                # sequence steps per DMA/exp chunk
BSC_BITS = 0              # gamma-side downscale (unneeded with MU=4.9)
LN_BITS = 16              # Ln inputs scaled by 2^-16 (ACT Ln range limit)
NSL = 512                 # numerator gpsimd slice width (columns)
ESL = 2048                # ACT exp slice width (columns)
P_BUFS = 4                # chain-state tile rotation depth
Q_BUFS = 2                # matmul PSUM tile rotation depth

_CACHE = {}


def _build(s=S, bl=BL, chunk=CHUNK, renorm_r=R):
    import contextlib
    import math
    import concourse.bacc as bacc
    import concourse.mybir as mybir
    import concourse.tile as tile
    from concourse._compat import axon_active

    fp32 = mybir.dt.float32
    bf16 = mybir.dt.bfloat16
    Alu = mybir.AluOpType
    Act = mybir.ActivationFunctionType

    nc = bacc.Bacc(
        "TRN2",
        target_bir_lowering=False,
        debug=not axon_active(),
        num_devices=NCORES,
    )

    half = s // 2
    assert half % chunk == 0
    n_ch = half // chunk
    nsteps = half - 1         # per-chain scan steps (k = 1..nsteps)
    gw = bl // NG

    emB = nc.dram_tensor("emB", [P2, half * bl], bf16, kind="ExternalInput")
    maskC = nc.dram_tensor("maskC", [P2, half * bl], bf16, kind="ExternalInput")
    # stage0: cols 0:48 = transT on rows 0:48 / transR on rows 64:112;
    # col 48 = [start | -inf pad | end] -- one DMA covers W2/WbV/eSE inputs
    stage0 = nc.dram_tensor("stage0", [P2, T + 1], fp32, kind="ExternalInput")
    # stage1: histP on rows 64:112 cols 0:48 (pairs with stage0's transR);
    # rows 0:48: col 48 = startv, 49 = hist0, 50 = endv, 51 = histN
    stage1 = nc.dram_tensor("stage1", [P2, T + 4], fp32, kind="ExternalInput")
    selmat = nc.dram_tensor("selmat", [P2, 2], bf16, kind="ExternalInput")
    selmatT = nc.dram_tensor("selmatT", [2, P2], bf16, kind="ExternalInput")
    denom_out = nc.dram_tensor("denom_out", [1, bl], fp32, kind="ExternalOutput")
    numer_out = nc.dram_tensor("numer_out", [1, 1], fp32, kind="ExternalOutput")

    rn = [k for k in range(renorm_r, nsteps, renorm_r)]
    rn_set = set(rn)
    nr = 2 * len(rn)          # each renorm event records F and B ln z rows

    with tile.TileContext(nc) as tc:
        with contextlib.ExitStack() as ctx:
            const = ctx.enter_context(tc.tile_pool(name="const", bufs=1))
            work = ctx.enter_context(tc.tile_pool(name="work", bufs=1))
            psum = ctx.enter_context(tc.tile_pool(name="psum", bufs=1, space="PSUM"))

            # ---- chunk-0 emissions DMA first: it gates the chain start.
            # Split so the first exp slice only waits for the head piece.
            fw0 = chunk * bl
            HD = 256      # tiny head piece so the chain starts early
            emb0 = const.tile([P2, fw0], bf16, tag="emb", bufs=2)
            nc.sync.dma_start(emb0[:, 0:HD], emB[:, 0:HD])
            st0 = const.tile([P2, T + 1], fp32)
            nc.sync.dma_start(st0[:], stage0[:, :])
            nc.sync.dma_start(emb0[:, HD:ESL], emB[:, HD:ESL])
            nc.sync.dma_start(emb0[:, ESL:fw0], emB[:, ESL:fw0])

            # ---- constants / parameters ----
            neg_mu = const.tile([P2, 1], fp32)
            nc.vector.memset(neg_mu[:], -float(MU))

            # W2 = blockdiag(exp(transT - MU) at [0:T], exp(transR - MU) at
            # [OFF:P2]) -- one stationary advances both chains (bf16)
            W2 = const.tile([P2, P2], bf16)
            nc.vector.memset(W2[:], 0.0)
            nc.scalar.activation(W2[0:T, 0:T], st0[0:T, 0:T], Act.Exp,
                                 bias=neg_mu[0:T, :])
            nc.scalar.activation(W2[OFF:P2, OFF:P2], st0[OFF:P2, 0:T],
                                 Act.Exp, bias=neg_mu[OFF:P2, :])

            # vertical [0; 0; Wb] so the final beta matmul reads full-span
            # APs (partition-offset operands are unreliable on HW)
            WbV = const.tile([P2, T], bf16)
            nc.vector.memset(WbV[:], 0.0)
            nc.scalar.activation(WbV[OFF:P2, 0:T], st0[OFF:P2, 0:T],
                                 Act.Exp, bias=neg_mu[OFF:P2, :])

            # combined init column: exp([start | -inf | end])
            eSE = const.tile([P2, 1], fp32)
            nc.scalar.activation(eSE[:], st0[:, T:T + 1], Act.Exp)

            # 0/1 selector matrices for the renorm column-sum (bf16, host-built)
            sel_sb = const.tile([P2, 2], bf16)
            nc.sync.dma_start(sel_sb[:], selmat[:, :])
            selT_sb = const.tile([2, P2], bf16)
            nc.sync.dma_start(selT_sb[:], selmatT[:, :])
            ones_k = const.tile([T, 1], fp32)
            nc.vector.memset(ones_k[:], 1.0)
            ones_2 = const.tile([2, 1], fp32)
            nc.vector.memset(ones_2[:], 1.0)

            # gpsimd emission-select accumulator: wide fp32 rows, summed once
            # at the end (Pool has no free-axis reduce).  Column NSL is a
            # guard slot (see finalize).
            naccW = work.tile([P2, NSL + 1], fp32)
            nc.gpsimd.memset(naccW[:], 0.0)

            zbuf = work.tile([2, bl, max(len(rn), 1)], fp32)

            # per-group chain state
            gp = [None] * NG
            g_pend = [None] * NG
            g_pend_at = [-1] * NG
            g_ri = [0] * NG

            def chunk_setup(ci):
                i0 = ci * chunk
                fw = chunk * bl
                if ci == 0:
                    emb = emb0    # DMA'd at program start (split head piece)
                else:
                    emb = const.tile([P2, fw], bf16, tag="emb", bufs=2)
                    nc.sync.dma_start(emb[:], emB[:, i0 * bl:(i0 + chunk) * bl])
                # exp(em) in ACT slices so the chain can consume early columns
                ech = const.tile([P2, fw], fp32, tag="ech", bufs=2)
                bounds = ([0, HD] if ci == 0 else [0]) + \
                    list(range(ESL, fw, ESL)) + [fw]
                for e0, e1 in zip(bounds[:-1], bounds[1:]):
                    nc.scalar.activation(ech[:, e0:e1], emb[:, e0:e1], Act.Exp)
                msk = const.tile([P2, fw], bf16, tag="msk", bufs=2)
                nc.sync.dma_start(msk[:], maskC[:, i0 * bl:(i0 + chunk) * bl])
                # numerator: gpsimd select-sum via the one-hot mask
                for s0 in range(0, fw, NSL):
                    nprod = const.tile([P2, NSL], bf16, tag="nprod", bufs=2)
                    nc.gpsimd.tensor_mul(nprod[:], msk[:, s0:s0 + NSL],
                                         emb[:, s0:s0 + NSL])
                    nc.gpsimd.tensor_add(naccW[:, 0:NSL], naccW[:, 0:NSL],
                                         nprod[:])
                return ech

            echs = {0: chunk_setup(0)}
            for ci in range(n_ch):
                i0 = ci * chunk
                ech = echs.pop(ci)
                if ci + 1 < n_ch:
                    echs[ci + 1] = chunk_setup(ci + 1)

                if ci == 0:
                    for g in range(NG):
                        p0 = const.tile([P2, gw], bf16, tag=f"p{g}",
                                        bufs=P_BUFS)
                        nc.vector.tensor_scalar_mul(
                            p0[:], ech[:, g * gw:(g + 1) * gw], eSE[:])
                        gp[g] = p0

                for j in range(chunk):
                    k = i0 + j
                    if k < 1 or k > nsteps:
                        continue
                    for g in range(NG):
                        esl = ech[:, j * bl + g * gw:j * bl + (g + 1) * gw]
                        if g_pend[g] is not None and k == g_pend_at[g]:
                            esl = g_pend[g][:]
                            g_pend[g] = None
                        q = psum.tile([P2, gw], fp32, tag=f"q{g}", bufs=Q_BUFS)
                        nc.tensor.matmul(q[:], W2[:], gp[g][:])
                        newp = const.tile([P2, gw], bf16, tag=f"p{g}",
                                          bufs=P_BUFS)
                        nc.vector.tensor_mul(newp[:], q[:], esl)
                        gp[g] = newp

                        if k in rn_set:
                            z = psum.tile([2, gw], fp32, tag=f"z{g}", bufs=1)
                            nc.tensor.matmul(z[:], sel_sb[:], gp[g][:])
                            rv = const.tile([2, gw], bf16, tag=f"rv{g}",
                                            bufs=2)
                            with nc.allow_low_precision(
                                    reason="renorm scale; ln z recorded exactly"):
                                nc.vector.reciprocal(rv[:], z[:])
                            rbc = psum.tile([P2, gw], fp32, tag=f"rbc{g}",
                                            bufs=1)
                            nc.tensor.matmul(rbc[:], selT_sb[:], rv[:])
                            nc.scalar.activation(
                                zbuf[:, g * gw:(g + 1) * gw, g_ri[g]], z[:],
                                Act.Copy)
                            g_ri[g] += 1
                            # pre-scale the ech slice of step k+DEFER (same
                            # chunk: DEFER < chunk alignment) off the chain
                            ja = j + DEFER
                            esc = const.tile([P2, gw], fp32, tag=f"esc{g}",
                                             bufs=2)
                            nc.vector.tensor_mul(
                                esc[:],
                                ech[:, ja * bl + g * gw:ja * bl + (g + 1) * gw],
                                rbc[:])
                            g_pend[g] = esc
                            g_pend_at[g] = k + DEFER

            # ---- numerator finalize first (its output DMA overlaps the
            # denominator combine).  Parameter-table dot products vs host
            # histograms: issued after the scan so their DVE ops never
            # head-of-line block the chain multiplies; stage1 landed long ago.
            st1 = const.tile([P2, T + 4], fp32)
            nc.sync.dma_start(st1[:], stage1[:, :])
            nacc = work.tile([P2, 1], fp32)
            nc.vector.memset(nacc[:], 0.0)
            scr48 = work.tile([P2, T], fp32)
            na_p = work.tile([P2, 1], fp32)
            nc.vector.scalar_tensor_tensor(
                scr48[OFF:P2, :], st0[OFF:P2, 0:T], 0.0, st1[OFF:P2, 0:T],
                Alu.add, Alu.mult, accum_out=na_p[OFF:P2, :],
            )
            nc.vector.tensor_add(nacc[OFF:P2, :], nacc[OFF:P2, :],
                                 na_p[OFF:P2, :])
            scr1 = work.tile([T, 1], fp32)
            na_s = work.tile([T, 1], fp32)
            nc.vector.scalar_tensor_tensor(
                scr1[:], st1[0:T, T:T + 1], 0.0, st1[0:T, T + 1:T + 2],
                Alu.add, Alu.mult, accum_out=na_s[:],
            )
            nc.vector.tensor_add(nacc[0:T, :], nacc[0:T, :], na_s[:])
            scr2 = work.tile([T, 1], fp32)
            na_e = work.tile([T, 1], fp32)
            nc.vector.scalar_tensor_tensor(
                scr2[:], st1[0:T, T + 2:T + 3], 0.0, st1[0:T, T + 3:T + 4],
                Alu.add, Alu.mult, accum_out=na_e[:],
            )
            nc.vector.tensor_add(nacc[0:T, :], nacc[0:T, :], na_e[:])
            # Guard: write 0 into naccW's spare column from the final chain
            # state.  This makes the accumulation depend on the scan's end,
            # so the static scheduler cannot hoist it ahead of pending chain
            # work on its engine (an early-ready op in an in-order stream
            # would stall everything behind it).
            nc.vector.scalar_tensor_tensor(
                naccW[0:1, NSL:NSL + 1], gp[0][0:1, 0:1], 0.0,
                gp[0][0:1, 0:1], Alu.mult, Alu.mult)
            # free-axis sum of naccW on ACT (accum_out), off the DVE
            nacc_p = work.tile([P2, 1], fp32)
            nc.scalar.activation(naccW[:], naccW[:], Act.Copy,
                                 accum_out=nacc_p[:])
            nc.vector.tensor_add(nacc[:], nacc[:], nacc_p[:])
            onesp = const.tile([P2, 1], fp32)
            nc.vector.memset(onesp[:], 1.0)
            nz = psum.tile([1, 1], fp32, tag="z0", bufs=1)
            nc.tensor.matmul(nz[:], nacc[:], onesp[:])
            ns = work.tile([1, 1], fp32)
            nc.vector.tensor_copy(ns[:], nz[:])
            nc.sync.dma_start(numer_out[0:1, :], ns[:])

            # ---- finalize denominator ----
            # beta_cut = Wb^T gamma; Z = sum_t alpha * beta_cut * 2^-BSC
            ln_shift = LN_BITS * math.log(2.0)
            c_init = (float(MU) * (s - 1) + (nr + 1) * ln_shift
                      + BSC_BITS * math.log(2.0))
            pend = work.tile([T, bl], fp32)
            for g in range(NG):
                bq = psum.tile([P2, gw], fp32, tag=f"rbc{g}", bufs=1)
                nc.tensor.matmul(bq[0:T, :], WbV[:], gp[g][:])
                nc.vector.tensor_mul(pend[:, g * gw:(g + 1) * gw],
                                     gp[g][0:T, :], bq[0:T, :])
            fz = psum.tile([1, bl], fp32, tag="z0", bufs=1)
            nc.tensor.matmul(fz[:], ones_k[:], pend[:])
            lnf = work.tile([1, bl], fp32)
            nc.scalar.activation(lnf[:], fz[:], Act.Ln, scale=2.0 ** -LN_BITS)
            dn = work.tile([1, bl], fp32)
            if nr > 0:
                nrr = len(rn)
                nc.scalar.activation(zbuf[:, :, 0:nrr], zbuf[:, :, 0:nrr],
                                     Act.Ln, scale=2.0 ** -LN_BITS)
                lnsum2 = work.tile([2, bl], fp32)
                nc.vector.tensor_reduce(lnsum2[:], zbuf[:, :, 0:nrr],
                                        mybir.AxisListType.X, Alu.add)
                lnrow = psum.tile([1, bl], fp32, tag="z1", bufs=1)
                nc.tensor.matmul(lnrow[:], ones_2[:], lnsum2[:])
                # dn = (lnf + c_init) + lnrow in one DVE op
                nc.vector.scalar_tensor_tensor(
                    dn[:], lnf[:], float(c_init), lnrow[:],
                    Alu.add, Alu.add)
            else:
                nc.vector.tensor_scalar_add(dn[:], lnf[:], float(c_init))
            nc.sync.dma_start(denom_out[0:1, :], dn[:])

    nc.compile()
    return nc


def _get_nc():
    if "nc" not in _CACHE:
        _CACHE["nc"] = _build()
    return _CACHE["nc"]


def _merge_em(em_c, bl):
    """(bl, S, T) -> (P2, half*bl): rows 0-47 forward em (step j),
    rows 64-111 backward em (step S-1-j), dead rows zero."""
    s = em_c.shape[1]
    half = s // 2
    fwd = em_c[:, 0:half]                       # (bl, half, T)
    bwd = em_c[:, ::-1][:, 0:half]
    out = np.zeros((P2, half * bl), np.float32)
    out[0:T] = np.ascontiguousarray(fwd.transpose(2, 1, 0)).reshape(T, half * bl)
    out[OFF:P2] = np.ascontiguousarray(bwd.transpose(2, 1, 0)).reshape(T, half * bl)
    return out


def _merge_mask(tg_c, bl):
    """One-hot tag mask, same layout as _merge_em (P2, half*bl) bf16."""
    import ml_dtypes
    s = tg_c.shape[1]
    half = s // 2
    ar = np.arange(T, dtype=tg_c.dtype)[:, None, None]
    out = np.zeros((P2, half * bl), np.dtype(ml_dtypes.bfloat16))
    fwd = (tg_c[:, 0:half].T[None, :, :] == ar)          # (T, half, bl)
    bwd = (tg_c[:, ::-1][:, 0:half].T[None, :, :] == ar)
    out[0:T] = fwd.reshape(T, half * bl).astype(ml_dtypes.bfloat16)
    out[OFF:P2] = bwd.reshape(T, half * bl).astype(ml_dtypes.bfloat16)
    return out


def _host_prep(emissions, tags, transitions, start_transitions,
               end_transitions):
    import ml_dtypes
    transT = np.ascontiguousarray(transitions.T, dtype=np.float32)
    transR = np.ascontiguousarray(transitions, dtype=np.float32)
    st0 = np.zeros((P2, T + 1), np.float32)
    st0[0:T, 0:T] = transT
    st0[OFF:P2, 0:T] = transR
    st0[:, T] = -100.0                              # dead rows -> exp = 0
    st0[0:T, T] = start_transitions
    st0[OFF:P2, T] = end_transitions
    sel = np.zeros((P2, 2), np.dtype(ml_dtypes.bfloat16))
    sel[0:T, 0] = 1.0
    sel[OFF:P2, 1] = 1.0
    selT = np.ascontiguousarray(sel.T)

    in_maps = []
    for c in range(NCORES):
        sl = slice(c * BL, (c + 1) * BL)
        em_c = emissions[sl]                      # (BL, S, T)
        tg_c = tags[sl]                           # (BL, S) int32
        h0 = np.bincount(tg_c[:, 0], minlength=T).astype(np.float32)
        hN = np.bincount(tg_c[:, -1], minlength=T).astype(np.float32)
        pair = tg_c[:, 1:].astype(np.int64) * T + tg_c[:, :-1].astype(np.int64)
        hP = np.bincount(pair.ravel(), minlength=T * T).astype(np.float32).reshape(T, T)
        st1 = np.zeros((P2, T + 4), np.float32)
        st1[OFF:P2, 0:T] = hP
        st1[0:T, T] = start_transitions
        st1[0:T, T + 1] = h0
        st1[0:T, T + 2] = end_transitions
        st1[0:T, T + 3] = hN
        emc = _merge_em(em_c, BL)
        in_maps.append({
            "emB": emc.astype(ml_dtypes.bfloat16),
            "maskC": _merge_mask(tg_c, BL),
            "stage0": st0, "stage1": st1,
            "selmat": sel, "selmatT": selT,
        })
    return in_maps


def kernel(emissions, tags, mask, transitions, start_transitions,
           end_transitions):
    from concourse.bass_utils import run_bass_kernel_spmd

    emissions = np.asarray(emissions, dtype=np.float32)
    tags = np.asarray(tags, dtype=np.int32)
    transitions = np.asarray(transitions, dtype=np.float32)
    start_transitions = np.asarray(start_transitions, dtype=np.float32)
    end_transitions = np.asarray(end_transitions, dtype=np.float32)

    nc = _get_nc()
    in_maps = _host_prep(emissions, tags, transitions, start_transitions,
                         end_transitions)
    res = run_bass_kernel_spmd(nc, in_maps, core_ids=list(range(NCORES)))

    denom_sum = 0.0
    numer_sum = 0.0
    for r in res.results:
        denom_sum += float(np.asarray(r["denom_out"], dtype=np.float64).sum())
        numer_sum += float(np.asarray(r["numer_out"], dtype=np.float64).sum())
    loss = (denom_sum - numer_sum) / B
    return np.float32(loss)
